# revision 31
# baseline (speedup 1.0000x reference)
"""Trainium2 Bass kernel v3 for nn_FCPairedLayer (pairwise MLP edge scorer).

Math (B=2, N=1024, C=128, H1=128, H2=64):
    aTb1 = (x @ W1[:C] + b1).T per token; rT = (x @ W1[C:]).T per token
    H_i  = relu(rT[:, j] + aTb1[:, i])        [H1=128, w] per row i  (DVE 4x)
    ph   = W2.T @ H (2 i's col-stacked on PE) -> PSUM f32
    h2s  = relu(ph + b2)                      -> SBUF bf16 (ACT)
    y    = w3 . h2s                           -> PSUM (row-packed) -> SBUF -> DRAM
    (+b3 and triangular mask on host)

Work tiling (uniform across 8 cores; per-core data selects tokens):
  8 width classes w_k = 1024-128k, 32 slots (16 pairs) each; core c takes
  rows 128k+16c..+16 of both batches.  Diagonal-block overhang masked on host.
  Class order [4,0,5,1,6,2,3,7] sandwiches DVE-heavy narrow classes between
  ACT-bound wide ones so H-construction pre-runs; ends on the cheapest class.

v3 vs v2 (115.1us -> 108.6us):
  - Stage-3 packs y densely: per class, 16 pairs -> 4 PE column strips
    (v=p%4) x 4 row-pairs (m=p//4, rows 2m/2m+1) via zero-padded lhsT
    slices accumulating into one PSUM tile (start only at m==0); batches
    of 4 pairs issue adjacently for 4-way strip concurrency, deferred one
    tile so the PE FIFO never waits on ACT's h2s.
  - One dense [128,w] PSUM->SBUF copy per class (DVE CAST) replaces 4
    sparse [128,w] copies, and ONE DMA per class (rows 0:104, junk rows
    skipped on host) on the idle GpSimd queue replaces 4 Sync-queue DMAs.
  - _split_sync_waits leaves EventSemaphore multi-waits intact: the final
    barrier no longer explodes into a ~10us per-engine semaphore storm.

Engine balance (measured): DVE ~89us (256 H tensor_scalars at 4x + 8 y
casts), ACT ~89us (86 h2s relus at 1x PSUM-read + rT preamble), PE ~67us.
Both elementwise engines are within ~10% of their stream+overhead floor
for this dataflow; bf16 PSUM (TRN3) would be needed to go much lower.
"""

import os
import numpy as np
import ml_dtypes

B, N, C = 2, 1024, 128
H1, H2 = 128, 64
NCORES = 8
BF16 = ml_dtypes.bfloat16

WIDTHS = [1024 - 128 * k for k in range(8)]
# flat y layout: class k block = [v=4, r=8, w]
CLASS_OFF = np.cumsum([0] + [104 * w for w in WIDTHS])  # in elements
DM = [1, 1, 1, 1, 2, 2, 4, 8]  # pairs merged per ph tile (dm*w <= 1024)

LAST_PERF = {}


def _cfg(name, default):
    v = os.environ.get(name)
    if v is None:
        return set(default)
    return set(int(x) for x in v.split(",") if x != "")


def _split_sync_waits(bir_json, limit=1):
    """Walrus here accepts at most one sync-wait per instruction; move extra
    Tile-generated waits onto single-wait EventSemaphore carriers."""
    import json

    data = json.loads(bir_json)
    for f in data.get("functions", []):
        for blk in f.get("blocks", []):
            out = []
            for ins in blk.get("instructions", []):
                si = ins.get("sync_info")
                ow = (si or {}).get("on_wait") or []
                if len(ow) > limit and ins.get("opcode") != "EventSemaphore":
                    for k, w in enumerate(ow[:-limit]):
                        out.append({
                            "debug": ins.get("debug", 0),
                            "engine": ins["engine"],
                            "name": f"{ins['name']}-xw{k}",
                            "opcode": "EventSemaphore",
                            "sync_info": {"on_update": [], "on_wait": [w]},
                        })
                    si["on_wait"] = ow[-limit:]
                out.append(ins)
            blk["instructions"] = out
    return json.dumps(data).encode()


def _install_compile_patch():
    import concourse.bass_utils as bu
    import concourse.bass2jax as b2j

    if getattr(bu, "_fc_split_waits_patch", False):
        return
    orig = bu.compile_bir_kernel

    def patched(bir_json, tmpdir, neff_name="file.neff"):
        return orig(_split_sync_waits(bir_json), tmpdir, neff_name)

    bu._fc_split_waits_patch = True
    bu.compile_bir_kernel = patched
    b2j.compile_bir_kernel = patched


def _build_program():
    import concourse.bass as bass
    import concourse.mybir as mybir
    from concourse.tile import TileContext

    f32 = mybir.dt.float32
    f32r = mybir.dt.float32r
    bf16 = mybir.dt.bfloat16

    # classes whose H construct runs on ACT (rest on DVE)
    act_h = _cfg("FC_ACT_H", [])
    # classes whose h2s relu runs on ACT (rest on DVE)
    act_h2 = _cfg("FC_ACT_H2", [0, 1, 2, 3, 4, 5, 6, 7])
    # classes whose y copy runs on ACT (rest on DVE)
    act_y = _cfg("FC_ACT_Y", [])
    # classes whose e=1 H slots run on ACT (DVE keeps e=0) - narrow classes
    act_h_half = _cfg("FC_ACT_H_HALF", [])
    # per-class h2s tile indices forced onto DVE (balance ACT-bound phases)
    _dve_h2 = os.environ.get("FC_DVE_H2", "")
    dve_h2 = set()
    for tok in _dve_h2.split(","):
        if tok:
            a, b = tok.split(":")
            dve_h2.add((int(a), int(b)))

    nc = bass.Bass()
    xr_d = nc.declare_dram_parameter("xr", [C, 256], f32r, isOutput=False)
    xw_d = nc.declare_dram_parameter("xw", [C, 2048], f32r, isOutput=False)
    w1l_d = nc.declare_dram_parameter("w1l", [C, H1], f32r, isOutput=False)
    w1r_d = nc.declare_dram_parameter("w1r", [C, H1], f32r, isOutput=False)
    b1c_d = nc.declare_dram_parameter("b1c", [H1, 1], f32, isOutput=False)
    w2b_d = nc.declare_dram_parameter("w2b", [H1, H2], bf16, isOutput=False)
    b2s_d = nc.declare_dram_parameter("b2s", [128, 1], f32, isOutput=False)
    w3m_d = nc.declare_dram_parameter("w3m", [128, 32], bf16, isOutput=False)
    y_d = nc.declare_dram_parameter("y", [1, 479232], bf16, isOutput=True)

    Relu = mybir.ActivationFunctionType.Relu
    ADD = mybir.AluOpType.add
    MAX = mybir.AluOpType.max

    with TileContext(nc) as tc:
        with tc.tile_pool(name="const", bufs=1) as const:
            w1l_t = const.tile([C, H1], f32r, tag="w1l")
            w1r_t = const.tile([C, H1], f32r, tag="w1r")
            b1c_t = const.tile([H1, 1], f32, tag="b1c")
            w2b_t = const.tile([H1, H2], bf16, tag="w2b")
            b2s_t = const.tile([128, 1], f32, tag="b2s")
            w3m_t = const.tile([128, 32], bf16, tag="w3m")
            xr_t = const.tile([C, 256], f32r, tag="xr")
            xw_t = const.tile([C, 2048], f32r, tag="xw")
            aTb1_t = const.tile([H1, 256], f32, tag="aTb1")
            rT_t = const.tile([H1, 2048], bf16, tag="rT")

            nc.sync.dma_start(out=xw_t[:, 512:1024], in_=xw_d[:, 512:1024])
            nc.sync.dma_start(out=w1r_t, in_=w1r_d[:])
            nc.sync.dma_start(out=xr_t, in_=xr_d[:])
            nc.sync.dma_start(out=w1l_t, in_=w1l_d[:])
            for t, d in [(b1c_t, b1c_d), (w2b_t, w2b_d), (b2s_t, b2s_d),
                         (w3m_t, w3m_d)]:
                nc.sync.dma_start(out=t, in_=d[:])

            with tc.tile_pool(name="pre", bufs=2, space="PSUM") as pre:
                pa = pre.tile([128, 256], f32, tag="pa")
                nc.tensor.matmul(pa, lhsT=w1l_t, rhs=xr_t,
                                 start=True, stop=True)
                nc.vector.tensor_scalar(aTb1_t, pa, b1c_t, None, ADD)
                for ch in (1, 3, 0, 2):
                    if ch != 1:
                        nc.sync.dma_start(
                            out=xw_t[:, ch * 512:(ch + 1) * 512],
                            in_=xw_d[:, ch * 512:(ch + 1) * 512])
                    pr = pre.tile([128, 512], f32, tag="pr")
                    nc.tensor.matmul(pr, lhsT=w1r_t,
                                     rhs=xw_t[:, ch * 512:(ch + 1) * 512],
                                     start=True, stop=True)
                    nc.scalar.copy(rT_t[:, ch * 512:(ch + 1) * 512], pr)

            with (
                tc.tile_pool(name="Hp", bufs=16) as Hp,
                tc.tile_pool(name="h2p", bufs=10) as h2p,
                tc.tile_pool(name="ysbp", bufs=2) as ysbp,
                tc.tile_pool(name="php", bufs=3, space="PSUM") as php,
                tc.tile_pool(name="pyp", bufs=1, space="PSUM") as pyp,
            ):
                ready3 = []   # deferred stage-3 batches: (batch, py, w)
                pendY = []    # deferred y copy+DMA: (py, ysb, k)

                def _flush3(batch, py_, w_):
                    # issue 4 pairs' stage-3 MMs adjacently: one per
                    # 32-wide PE column strip so they run concurrently
                    nqy = (w_ + 511) // 512
                    for q in range(nqy):
                        lo = 512 * q
                        hi = min(512 * (q + 1), w_)
                        for (v, m, h2t, hoff) in batch:
                            nc.tensor.matmul(
                                py_[32 * v:32 * v + 8, lo:hi],
                                lhsT=w3m_t[:, 8 * m:8 * m + 8],
                                rhs=h2t[:, hoff + lo:hoff + hi],
                                start=(m == 0), stop=(m == 3),
                                tile_position=(0, 32 * v),
                                skip_group_check=True)

                def _emit_y(py_, ysb_, k_):
                    if k_ in act_y:
                        nc.scalar.copy(ysb_, py_)
                    else:
                        nc.vector.tensor_copy(ysb_, py_)
                    yv = y_d[0, CLASS_OFF[k_]:CLASS_OFF[k_ + 1]].rearrange(
                        "(r f) -> r f", r=104)
                    eng = nc.sync if k_ in (3, 7) else nc.gpsimd
                    eng.dma_start(out=yv, in_=ysb_[0:104, :])

                def _drain(limit):
                    # flush deferred stage-3 batches (their h2s results
                    # landed while later stage-2 matmuls kept PE busy),
                    # then any deferred y copy+DMA
                    while len(ready3) > limit:
                        b, py_, w_ = ready3.pop(0)
                        _flush3(b, py_, w_)
                    if not ready3 and pendY:
                        _emit_y(*pendY.pop(0))

                for k in [4, 0, 5, 1, 2, 6, 3, 7]:
                    w = WIDTHS[k]
                    dm = DM[k]
                    py = pyp.tile([128, w], f32, tag="py")
                    ysb = ysbp.tile([128, w], bf16, tag="ysb")
                    pend3 = []
                    for t in range(16 // dm):
                        Ht0 = Hp.tile([128, dm * w], bf16, tag="H0")
                        Ht1 = Hp.tile([128, dm * w], bf16, tag="H1")
                        Hts = [Ht0, Ht1]
                        for half in range(dm):
                            p = t * dm + half
                            b, rp = divmod(p, 8)
                            c0 = 32 * k + 16 * b + 2 * rp
                            off = 1024 * b + 128 * k
                            for e in range(2):
                                hd = Hts[e][:, half * w:(half + 1) * w]
                                if k in act_h or (k in act_h_half
                                                  and e == 1):
                                    nc.scalar.activation(
                                        hd, rT_t[:, off:off + w], Relu,
                                        bias=aTb1_t[:, c0 + e:c0 + e + 1])
                                else:
                                    nc.vector.tensor_scalar(
                                        hd, rT_t[:, off:off + w],
                                        aTb1_t[:, c0 + e:c0 + e + 1],
                                        0.0, ADD, op1=MAX)
                        mw = dm * w
                        nqm = (mw + 511) // 512
                        ph = php.tile([128, mw], f32, tag="ph")
                        for e in range(2):
                            for q in range(nqm):
                                qs = slice(512 * q, min(512 * (q + 1), mw))
                                nc.tensor.matmul(
                                    ph[64 * e:64 * (e + 1), qs],
                                    lhsT=w2b_t, rhs=Hts[e][:, qs],
                                    start=True, stop=True,
                                    tile_position=(0, 64 * e))
                        _drain(limit=1)
                        h2s = h2p.tile([128, mw], bf16, tag="h2s")
                        if (k, t) in dve_h2:
                            nc.vector.tensor_scalar(h2s, ph, b2s_t, 0.0,
                                                    ADD, op1=MAX)
                        elif k in act_h2:
                            nc.scalar.activation(h2s, ph, Relu, bias=b2s_t)
                        else:
                            nc.vector.tensor_scalar(h2s, ph, b2s_t, 0.0,
                                                    ADD, op1=MAX)
                        for half in range(dm):
                            p = t * dm + half
                            v, m = p % 4, p // 4
                            pend3.append((v, m, h2s, half * w))
                            if len(pend3) == 4:
                                ready3.append((pend3, py, w))
                                pend3 = []
                    pendY.append((py, ysb, k))
                while ready3 or pendY:
                    _drain(limit=0)
    return nc


def _pack_inputs(x, W1, b1, W2, b2, W3, b3):
    xT = np.ascontiguousarray(x.transpose(0, 2, 1)).astype(np.float32)
    w1l = np.ascontiguousarray(W1[:C]).astype(np.float32)
    w1r = np.ascontiguousarray(W1[C:]).astype(np.float32)
    b1c = np.ascontiguousarray(b1.reshape(H1, 1)).astype(np.float32)
    w2b = np.ascontiguousarray(W2).astype(BF16)
    b2s = np.concatenate([b2, b2]).reshape(128, 1).astype(np.float32)
    w3m = np.zeros((128, 32), dtype=BF16)
    for m in range(4):
        w3m[0:64, 8 * m + 2 * m] = W3[:, 0].astype(BF16)
        w3m[64:128, 8 * m + 2 * m + 1] = W3[:, 0].astype(BF16)
    xw = np.ascontiguousarray(np.concatenate([xT[0], xT[1]], axis=1))

    in_maps = []
    for c in range(NCORES):
        xr = np.empty((C, 256), dtype=np.float32)
        for k in range(8):
            for b in range(2):
                base = 128 * k + 16 * c
                xr[:, 32 * k + 16 * b:32 * k + 16 * b + 16] = \
                    xT[b][:, base:base + 16]
        in_maps.append({
            "xr": np.ascontiguousarray(xr), "xw": xw,
            "w1l": w1l, "w1r": w1r, "b1c": b1c, "w2b": w2b, "b2s": b2s,
            "w3m": w3m,
        })
    return in_maps


_TRIU = None


def _assemble(results, b3):
    global _TRIU
    y = np.zeros((B, N, N), dtype=np.float32)
    for c in range(NCORES):
        flat = results[c]["y"].reshape(-1).astype(np.float32)
        for k in range(8):
            w = WIDTHS[k]
            blk = flat[CLASS_OFF[k]:CLASS_OFF[k + 1]].reshape(104, w)
            for p in range(16):
                b, rp = divmod(p, 8)
                v, m = p % 4, p // 4
                for e in range(2):
                    i = 128 * k + 16 * c + 2 * rp + e
                    y[b, i, 128 * k:128 * k + w] = blk[32 * v + 2 * m + e]
    y += np.float32(b3[0])
    if _TRIU is None:
        _TRIU = np.triu(np.ones((N, N), dtype=np.float32), k=1)
    y *= _TRIU
    return y


def kernel(x, W1, b1, W2, b2, W3, b3):
    _install_compile_patch()
    from concourse.bass_utils import run_bass_kernel_spmd

    trace = bool(int(os.environ.get("FC_TRACE", "0")))
    nc = _build_program()
    in_maps = _pack_inputs(np.asarray(x), np.asarray(W1), np.asarray(b1),
                           np.asarray(W2), np.asarray(b2), np.asarray(W3),
                           np.asarray(b3))
    res = run_bass_kernel_spmd(nc, in_maps, core_ids=list(range(NCORES)),
                               trace=trace)
    LAST_PERF.clear()
    LAST_PERF.update({
        "exec_time_ns": res.exec_time_ns,
        "mean_exec_time_ns": res.mean_exec_time_ns,
        "trace": res.instructions_and_trace[1] if res.instructions_and_trace else None,
    })
    return _assemble(res.results, np.asarray(b3))


# revision 32
# speedup vs baseline: 1.0120x; 1.0120x over previous
"""Trainium2 Bass kernel v3 for nn_FCPairedLayer (pairwise MLP edge scorer).

Math (B=2, N=1024, C=128, H1=128, H2=64):
    aTb1 = (x @ W1[:C] + b1).T per token; rT = (x @ W1[C:]).T per token
    H_i  = relu(rT[:, j] + aTb1[:, i])        [H1=128, w] per row i  (DVE 4x)
    ph   = W2.T @ H (2 i's col-stacked on PE) -> PSUM f32
    h2s  = relu(ph + b2)                      -> SBUF bf16 (ACT)
    y    = w3 . h2s                           -> PSUM (row-packed) -> SBUF -> DRAM
    (+b3 and triangular mask on host)

Work tiling (uniform across 8 cores; per-core data selects tokens):
  8 width classes w_k = 1024-128k, 32 slots (16 pairs) each; core c takes
  rows 128k+16c..+16 of both batches.  Diagonal-block overhang masked on host.
  Class order [4,0,5,1,6,2,3,7] sandwiches DVE-heavy narrow classes between
  ACT-bound wide ones so H-construction pre-runs; ends on the cheapest class.

v3 vs v2 (115.1us -> 108.6us):
  - Stage-3 packs y densely: per class, 16 pairs -> 4 PE column strips
    (v=p%4) x 4 row-pairs (m=p//4, rows 2m/2m+1) via zero-padded lhsT
    slices accumulating into one PSUM tile (start only at m==0); batches
    of 4 pairs issue adjacently for 4-way strip concurrency, deferred one
    tile so the PE FIFO never waits on ACT's h2s.
  - One dense [128,w] PSUM->SBUF copy per class (DVE CAST) replaces 4
    sparse [128,w] copies, and ONE DMA per class (rows 0:104, junk rows
    skipped on host) on the idle GpSimd queue replaces 4 Sync-queue DMAs.
  - _split_sync_waits leaves EventSemaphore multi-waits intact: the final
    barrier no longer explodes into a ~10us per-engine semaphore storm.

Engine balance (measured): DVE ~89us (256 H tensor_scalars at 4x + 8 y
casts), ACT ~89us (86 h2s relus at 1x PSUM-read + rT preamble), PE ~67us.
Both elementwise engines are within ~10% of their stream+overhead floor
for this dataflow; bf16 PSUM (TRN3) would be needed to go much lower.
"""

import os
import numpy as np
import ml_dtypes

B, N, C = 2, 1024, 128
H1, H2 = 128, 64
NCORES = 8
BF16 = ml_dtypes.bfloat16

WIDTHS = [1024 - 128 * k for k in range(8)]
# flat y layout: class k block = [v=4, r=8, w]
CLASS_OFF = np.cumsum([0] + [104 * w for w in WIDTHS])  # in elements
DM = [1, 1, 1, 1, 2, 2, 4, 8]  # pairs merged per ph tile (dm*w <= 1024)

LAST_PERF = {}


def _cfg(name, default):
    v = os.environ.get(name)
    if v is None:
        return set(default)
    return set(int(x) for x in v.split(",") if x != "")


def _split_sync_waits(bir_json, limit=1):
    """Walrus here accepts at most one sync-wait per instruction; move extra
    Tile-generated waits onto single-wait EventSemaphore carriers."""
    import json

    data = json.loads(bir_json)
    for f in data.get("functions", []):
        for blk in f.get("blocks", []):
            out = []
            for ins in blk.get("instructions", []):
                si = ins.get("sync_info")
                ow = (si or {}).get("on_wait") or []
                if len(ow) > limit and ins.get("opcode") != "EventSemaphore":
                    for k, w in enumerate(ow[:-limit]):
                        out.append({
                            "debug": ins.get("debug", 0),
                            "engine": ins["engine"],
                            "name": f"{ins['name']}-xw{k}",
                            "opcode": "EventSemaphore",
                            "sync_info": {"on_update": [], "on_wait": [w]},
                        })
                    si["on_wait"] = ow[-limit:]
                out.append(ins)
            blk["instructions"] = out
    return json.dumps(data).encode()


def _install_compile_patch():
    import concourse.bass_utils as bu
    import concourse.bass2jax as b2j

    if getattr(bu, "_fc_split_waits_patch", False):
        return
    orig = bu.compile_bir_kernel

    def patched(bir_json, tmpdir, neff_name="file.neff"):
        return orig(_split_sync_waits(bir_json), tmpdir, neff_name)

    bu._fc_split_waits_patch = True
    bu.compile_bir_kernel = patched
    b2j.compile_bir_kernel = patched


def _build_program():
    import concourse.bass as bass
    import concourse.mybir as mybir
    from concourse.tile import TileContext

    f32 = mybir.dt.float32
    f32r = mybir.dt.float32r
    bf16 = mybir.dt.bfloat16

    # classes whose H construct runs on ACT (rest on DVE)
    act_h = _cfg("FC_ACT_H", [])
    # classes whose h2s relu runs on ACT (rest on DVE)
    act_h2 = _cfg("FC_ACT_H2", [0, 1, 2, 3, 4, 5, 6, 7])
    # classes whose y copy runs on ACT (rest on DVE)
    act_y = _cfg("FC_ACT_Y", [])
    # classes whose e=1 H slots run on ACT (DVE keeps e=0) - narrow classes
    act_h_half = _cfg("FC_ACT_H_HALF", [])
    # per-class h2s tile indices forced onto DVE (balance ACT-bound phases)
    _dve_h2 = os.environ.get("FC_DVE_H2", "")
    dve_h2 = set()
    for tok in _dve_h2.split(","):
        if tok:
            a, b = tok.split(":")
            dve_h2.add((int(a), int(b)))

    nc = bass.Bass()
    xr_d = nc.declare_dram_parameter("xr", [C, 256], f32r, isOutput=False)
    xw_d = nc.declare_dram_parameter("xw", [C, 2048], f32r, isOutput=False)
    w1l_d = nc.declare_dram_parameter("w1l", [C, H1], f32r, isOutput=False)
    w1r_d = nc.declare_dram_parameter("w1r", [C, H1], f32r, isOutput=False)
    b1c_d = nc.declare_dram_parameter("b1c", [H1, 1], f32, isOutput=False)
    w2b_d = nc.declare_dram_parameter("w2b", [H1, H2], bf16, isOutput=False)
    b2s_d = nc.declare_dram_parameter("b2s", [128, 1], f32, isOutput=False)
    w3m_d = nc.declare_dram_parameter("w3m", [128, 32], bf16, isOutput=False)
    y_d = nc.declare_dram_parameter("y", [1, 479232], bf16, isOutput=True)

    Relu = mybir.ActivationFunctionType.Relu
    ADD = mybir.AluOpType.add
    MAX = mybir.AluOpType.max

    with TileContext(nc) as tc:
        with tc.tile_pool(name="const", bufs=1) as const:
            w1l_t = const.tile([C, H1], f32r, tag="w1l")
            w1r_t = const.tile([C, H1], f32r, tag="w1r")
            b1c_t = const.tile([H1, 1], f32, tag="b1c")
            w2b_t = const.tile([H1, H2], bf16, tag="w2b")
            b2s_t = const.tile([128, 1], f32, tag="b2s")
            w3m_t = const.tile([128, 32], bf16, tag="w3m")
            xr_t = const.tile([C, 256], f32r, tag="xr")
            xw_t = const.tile([C, 2048], f32r, tag="xw")
            aTb1_t = const.tile([H1, 256], f32, tag="aTb1")
            rT_t = const.tile([H1, 2048], bf16, tag="rT")

            nc.sync.dma_start(out=xw_t[:, 512:1024], in_=xw_d[:, 512:1024])
            nc.sync.dma_start(out=w1r_t, in_=w1r_d[:])
            nc.sync.dma_start(out=xr_t, in_=xr_d[:])
            nc.sync.dma_start(out=w1l_t, in_=w1l_d[:])
            for t, d in [(b1c_t, b1c_d), (w2b_t, w2b_d), (b2s_t, b2s_d),
                         (w3m_t, w3m_d)]:
                nc.sync.dma_start(out=t, in_=d[:])

            with tc.tile_pool(name="pre", bufs=2, space="PSUM") as pre:
                pa = pre.tile([128, 256], f32, tag="pa")
                nc.tensor.matmul(pa, lhsT=w1l_t, rhs=xr_t,
                                 start=True, stop=True)
                nc.vector.tensor_scalar(aTb1_t, pa, b1c_t, None, ADD)
                for ch in (1, 3, 0, 2):
                    if ch != 1:
                        nc.sync.dma_start(
                            out=xw_t[:, ch * 512:(ch + 1) * 512],
                            in_=xw_d[:, ch * 512:(ch + 1) * 512])
                    pr = pre.tile([128, 512], f32, tag="pr")
                    nc.tensor.matmul(pr, lhsT=w1r_t,
                                     rhs=xw_t[:, ch * 512:(ch + 1) * 512],
                                     start=True, stop=True)
                    nc.scalar.copy(rT_t[:, ch * 512:(ch + 1) * 512], pr)

            with (
                tc.tile_pool(name="Hp", bufs=16) as Hp,
                tc.tile_pool(name="h2p", bufs=10) as h2p,
                tc.tile_pool(name="ysbp", bufs=2) as ysbp,
                tc.tile_pool(name="php", bufs=3, space="PSUM") as php,
                tc.tile_pool(name="pyp", bufs=1, space="PSUM") as pyp,
            ):
                ready3 = []   # deferred stage-3 batches: (batch, py, w)
                pendY = []    # deferred y copy+DMA: (py, ysb, k)

                def _flush3(batch, py_, w_):
                    # issue 4 pairs' stage-3 MMs adjacently: one per
                    # 32-wide PE column strip so they run concurrently
                    nqy = (w_ + 511) // 512
                    for q in range(nqy):
                        lo = 512 * q
                        hi = min(512 * (q + 1), w_)
                        for (v, m, h2t, hoff) in batch:
                            nc.tensor.matmul(
                                py_[32 * v:32 * v + 8, lo:hi],
                                lhsT=w3m_t[:, 8 * m:8 * m + 8],
                                rhs=h2t[:, hoff + lo:hoff + hi],
                                start=(m == 0), stop=(m == 3),
                                tile_position=(0, 32 * v),
                                skip_group_check=True)

                def _emit_y(py_, ysb_, k_):
                    if k_ in act_y:
                        nc.scalar.copy(ysb_, py_)
                    else:
                        nc.vector.tensor_copy(ysb_, py_)
                    yv = y_d[0, CLASS_OFF[k_]:CLASS_OFF[k_ + 1]].rearrange(
                        "(r f) -> r f", r=104)
                    nc.gpsimd.dma_start(out=yv, in_=ysb_[0:104, :])

                def _drain(limit):
                    # flush deferred stage-3 batches (their h2s results
                    # landed while later stage-2 matmuls kept PE busy),
                    # then any deferred y copy+DMA
                    while len(ready3) > limit:
                        b, py_, w_ = ready3.pop(0)
                        _flush3(b, py_, w_)
                    if not ready3 and pendY:
                        _emit_y(*pendY.pop(0))

                for k in [4, 0, 5, 1, 6, 2, 3, 7]:
                    w = WIDTHS[k]
                    dm = DM[k]
                    py = pyp.tile([128, w], f32, tag="py")
                    ysb = ysbp.tile([128, w], bf16, tag="ysb")
                    pend3 = []
                    for t in range(16 // dm):
                        Ht0 = Hp.tile([128, dm * w], bf16, tag="H0")
                        Ht1 = Hp.tile([128, dm * w], bf16, tag="H1")
                        Hts = [Ht0, Ht1]
                        for half in range(dm):
                            p = t * dm + half
                            b, rp = divmod(p, 8)
                            c0 = 32 * k + 16 * b + 2 * rp
                            off = 1024 * b + 128 * k
                            for e in range(2):
                                hd = Hts[e][:, half * w:(half + 1) * w]
                                if k in act_h or (k in act_h_half
                                                  and e == 1):
                                    nc.scalar.activation(
                                        hd, rT_t[:, off:off + w], Relu,
                                        bias=aTb1_t[:, c0 + e:c0 + e + 1])
                                else:
                                    nc.vector.tensor_scalar(
                                        hd, rT_t[:, off:off + w],
                                        aTb1_t[:, c0 + e:c0 + e + 1],
                                        0.0, ADD, op1=MAX)
                        mw = dm * w
                        nqm = (mw + 511) // 512
                        ph = php.tile([128, mw], f32, tag="ph")
                        for e in range(2):
                            for q in range(nqm):
                                qs = slice(512 * q, min(512 * (q + 1), mw))
                                nc.tensor.matmul(
                                    ph[64 * e:64 * (e + 1), qs],
                                    lhsT=w2b_t, rhs=Hts[e][:, qs],
                                    start=True, stop=True,
                                    tile_position=(0, 64 * e))
                        _drain(limit=1)
                        h2s = h2p.tile([128, mw], bf16, tag="h2s")
                        if (k, t) in dve_h2:
                            nc.vector.tensor_scalar(h2s, ph, b2s_t, 0.0,
                                                    ADD, op1=MAX)
                        elif k in act_h2:
                            nc.scalar.activation(h2s, ph, Relu, bias=b2s_t)
                        else:
                            nc.vector.tensor_scalar(h2s, ph, b2s_t, 0.0,
                                                    ADD, op1=MAX)
                        for half in range(dm):
                            p = t * dm + half
                            v, m = p % 4, p // 4
                            pend3.append((v, m, h2s, half * w))
                            if len(pend3) == 4:
                                ready3.append((pend3, py, w))
                                pend3 = []
                    pendY.append((py, ysb, k))
                while ready3 or pendY:
                    _drain(limit=0)
    return nc


def _pack_inputs(x, W1, b1, W2, b2, W3, b3):
    xT = np.ascontiguousarray(x.transpose(0, 2, 1)).astype(np.float32)
    w1l = np.ascontiguousarray(W1[:C]).astype(np.float32)
    w1r = np.ascontiguousarray(W1[C:]).astype(np.float32)
    b1c = np.ascontiguousarray(b1.reshape(H1, 1)).astype(np.float32)
    w2b = np.ascontiguousarray(W2).astype(BF16)
    b2s = np.concatenate([b2, b2]).reshape(128, 1).astype(np.float32)
    w3m = np.zeros((128, 32), dtype=BF16)
    for m in range(4):
        w3m[0:64, 8 * m + 2 * m] = W3[:, 0].astype(BF16)
        w3m[64:128, 8 * m + 2 * m + 1] = W3[:, 0].astype(BF16)
    xw = np.ascontiguousarray(np.concatenate([xT[0], xT[1]], axis=1))

    in_maps = []
    for c in range(NCORES):
        xr = np.empty((C, 256), dtype=np.float32)
        for k in range(8):
            for b in range(2):
                base = 128 * k + 16 * c
                xr[:, 32 * k + 16 * b:32 * k + 16 * b + 16] = \
                    xT[b][:, base:base + 16]
        in_maps.append({
            "xr": np.ascontiguousarray(xr), "xw": xw,
            "w1l": w1l, "w1r": w1r, "b1c": b1c, "w2b": w2b, "b2s": b2s,
            "w3m": w3m,
        })
    return in_maps


_TRIU = None


def _assemble(results, b3):
    global _TRIU
    y = np.zeros((B, N, N), dtype=np.float32)
    for c in range(NCORES):
        flat = results[c]["y"].reshape(-1).astype(np.float32)
        for k in range(8):
            w = WIDTHS[k]
            blk = flat[CLASS_OFF[k]:CLASS_OFF[k + 1]].reshape(104, w)
            for p in range(16):
                b, rp = divmod(p, 8)
                v, m = p % 4, p // 4
                for e in range(2):
                    i = 128 * k + 16 * c + 2 * rp + e
                    y[b, i, 128 * k:128 * k + w] = blk[32 * v + 2 * m + e]
    y += np.float32(b3[0])
    if _TRIU is None:
        _TRIU = np.triu(np.ones((N, N), dtype=np.float32), k=1)
    y *= _TRIU
    return y


def kernel(x, W1, b1, W2, b2, W3, b3):
    _install_compile_patch()
    from concourse.bass_utils import run_bass_kernel_spmd

    trace = bool(int(os.environ.get("FC_TRACE", "0")))
    nc = _build_program()
    in_maps = _pack_inputs(np.asarray(x), np.asarray(W1), np.asarray(b1),
                           np.asarray(W2), np.asarray(b2), np.asarray(W3),
                           np.asarray(b3))
    res = run_bass_kernel_spmd(nc, in_maps, core_ids=list(range(NCORES)),
                               trace=trace)
    LAST_PERF.clear()
    LAST_PERF.update({
        "exec_time_ns": res.exec_time_ns,
        "mean_exec_time_ns": res.mean_exec_time_ns,
        "trace": res.instructions_and_trace[1] if res.instructions_and_trace else None,
    })
    return _assemble(res.results, np.asarray(b3))
